# revision 8
# baseline (speedup 1.0000x reference)
# Gaussian-kernel ridge-regression matvec on 8 Trainium2 cores.
#
#   out_i = sum_j exp(-||x_i - y_j||^2 / g) * alpha_j
#   N=8192 queries, M=16384 train points, DIM=32, g scalar.
#
# Factorization (host prep is O(N+M), device does the O(N*M) part):
#   exp(-(x^2+y^2-2xy)/g)*a_j = exp(-x_i^2/g) * sign(a_j) * exp(s_ij),
#   s_ij = (2/g) x_i.y_j + c_j,   c_j = -y_j^2/g + ln|a_j|
# Train points are host-sorted so sign(a)>0 comes first (npos). Row scale
# exp(-x_i^2/g) is applied on host.
#
# fp16 hi/lo precision for x packed into a SINGLE K=66 matmul (PE
# streaming cost is per moving column, independent of contraction depth):
#   partitions  0-31: xh . yh
#   partitions 32-63: xl . yh   (yh replicated on-chip by the idle DVE;
#                                compute-engine partition bases must be
#                                32-aligned, hence this layout)
#   partitions 64-65: 1 . c_hi + 1 . c_lo   (c at full precision)
# Only the xh.yl rounding-noise term is dropped (~1e-3 absolute in s,
# ~0.1% relative noise in the output, far inside the tolerance).  Total
# input DMA is 1.25MB (yh rows + c rows + x), so the startup ramp is
# short and the two hardware DGE queues (SP + ACT) stay ahead of compute.
#
# ACT is the critical engine (1 elem/lane/cycle at 1.2 GHz over 16.8M
# elems/core ~ 110us): exactly one exp ACTIVATE per 4-bank PSUM group
# (FD=2048) with accum_out row sums; the one group containing the
# pos/neg boundary gets a DVE-side correction reduce
# (out = pos - neg = sum(parts) - 2*minority_part_of_split_group).

import numpy as np

N, M, DIM, NCORES = 8192, 16384, 32, 8
NLOC = N // NCORES
ITILES = NLOC // 128
GRP = 2048
NGRP = M // GRP
KPK = 2 * DIM + 2       # 66

_cache = {}


def _build(npos):
    import concourse.bass as bass
    import concourse.tile as tile
    from concourse import bacc, mybir

    f32 = mybir.dt.float32
    f16 = mybir.dt.float16
    Exp = mybir.ActivationFunctionType.Exp
    X = mybir.AxisListType.X

    nc = bacc.Bacc("TRN2", target_bir_lowering=False, debug=False)
    yhd = nc.dram_tensor("yhd", [DIM, M], f16, kind="ExternalInput").ap()
    cd = nc.dram_tensor("cd", [2, M], f16, kind="ExternalInput").ap()
    xpk = nc.dram_tensor("xpk", [KPK, NLOC], f16, kind="ExternalInput").ap()
    o = nc.dram_tensor("o", [128, ITILES], f32, kind="ExternalOutput").ap()

    gsplit, b0 = divmod(npos, GRP)

    with tile.TileContext(nc) as tc:
        with tc.tile_pool(name="ypool", bufs=1) as ypool, \
             tc.tile_pool(name="xpool", bufs=1) as xpool, \
             tc.tile_pool(name="psum", bufs=2, space="PSUM") as pp, \
             tc.tile_pool(name="parts", bufs=ITILES) as partp, \
             tc.tile_pool(name="small", bufs=5 * ITILES + 2) as smallp, \
             tc.tile_pool(name="res", bufs=1) as resp:

            # DMAs issued in consumption order, alternating between the
            # two hardware DGE queues (SP + ACT) so transfers land in
            # parallel and ahead of compute; 4 yh chunks of 4096 keep the
            # DMA (and semaphore) count low — the framework postamble
            # clears every allocated semaphore one by one
            YCH = 4096
            NYCH = M // YCH
            xt = xpool.tile([KPK, NLOC], f16, tag="xpk")
            yb = ypool.tile([KPK, M], f16, tag="yb")
            nc.sync.dma_start(yb[0:DIM, bass.ts(0, YCH)], yhd[:, bass.ts(0, YCH)])
            nc.scalar.dma_start(yb[2 * DIM:KPK], cd[:])
            nc.scalar.dma_start(xt[:], xpk[:])
            nc.scalar.dma_start(yb[0:DIM, bass.ts(1, YCH)], yhd[:, bass.ts(1, YCH)])
            nc.sync.dma_start(yb[0:DIM, bass.ts(2, YCH)], yhd[:, bass.ts(2, YCH)])
            nc.scalar.dma_start(yb[0:DIM, bass.ts(3, YCH)], yhd[:, bass.ts(3, YCH)])
            dummyw = smallp.tile([DIM, 1], f16, tag="dummyw")
            nc.vector.memset(dummyw[:], 0.0)
            # early dummy exp so the ~2.7us ACT table load overlaps DMA
            warm = smallp.tile([1, 1], f32, tag="warm")
            nc.scalar.activation(warm[:], dummyw[0:1, 0:1], Exp)
            # tiny self-copy: a DVE op whose only job is to absorb the
            # c-row DMA wait so matmuls only carry DVE + psum-release sems
            nc.vector.tensor_copy(yb[2 * DIM:KPK, 0:1], yb[2 * DIM:KPK, 0:1])
            # replicate yh onto partitions 32-63 (DVE is otherwise idle)
            for ci in range(NYCH):
                nc.vector.tensor_copy(yb[DIM:2 * DIM, bass.ts(ci, YCH)],
                                      yb[0:DIM, bass.ts(ci, YCH)])
            # pre-touch x on the PE so no real matmul waits on its DMA
            dps = pp.tile([1, 2], f32, tag="ps")
            nc.tensor.matmul(dps[:, 0:1], dummyw[:], dummyw[:],
                             start=True, stop=True)
            nc.tensor.matmul(dps[:, 1:2], dummyw[:], xt[0:DIM, 0:1],
                             start=True, stop=True)

            res = resp.tile([128, ITILES], f32)

            for it in range(ITILES):
                xw = xt[:, bass.ts(it, 128)]
                parts = partp.tile([128, NGRP], f32, tag="parts")
                corr = None

                for gi in range(NGRP):
                    ps = pp.tile([128, GRP], f32, tag="ps")
                    if it == 0 and gi % 2 == 0:
                        # pre-touch this yh chunk: absorbs its DMA wait
                        nc.tensor.matmul(ps[0:1, 0:1], dummyw[:],
                                         yb[0:DIM, gi * GRP:gi * GRP + 1],
                                         start=True, stop=True)
                    for k in range(4):
                        nc.tensor.matmul(ps[:, bass.ts(k, 512)], xw,
                                         yb[:, gi * GRP + k * 512:
                                             gi * GRP + (k + 1) * 512],
                                         start=True, stop=True)
                    nc.scalar.activation(ps[:], ps[:], Exp,
                                         accum_out=parts[:, gi:gi + 1])
                    if gi == gsplit and b0 > 0:
                        corr = smallp.tile([128, 1], f32, tag="corr")
                        if b0 <= GRP // 2:
                            nc.vector.reduce_sum(corr[:], ps[:, 0:b0], axis=X)
                        else:
                            nc.vector.reduce_sum(corr[:], ps[:, b0:GRP], axis=X)

                # pos groups [0, pg), neg groups [pg, NGRP); the split
                # group counts toward whichever side its reduce was NOT on
                if b0 == 0:
                    pg = gsplit
                elif b0 <= GRP // 2:
                    pg = gsplit          # split group tallied as neg, corr=pos part
                else:
                    pg = gsplit + 1      # split group tallied as pos, corr=neg part

                possum = smallp.tile([128, 1], f32, tag="pos")
                negsum = smallp.tile([128, 1], f32, tag="neg")
                if pg:
                    nc.vector.reduce_sum(possum[:], parts[:, 0:pg], axis=X)
                else:
                    nc.vector.memset(possum[:], 0.0)
                if NGRP - pg:
                    nc.vector.reduce_sum(negsum[:], parts[:, pg:NGRP], axis=X)
                else:
                    nc.vector.memset(negsum[:], 0.0)
                if corr is None:
                    nc.vector.tensor_sub(res[:, it:it + 1], possum[:], negsum[:])
                else:
                    tmp = smallp.tile([128, 1], f32, tag="tmp")
                    tw = smallp.tile([128, 1], f32, tag="tw")
                    nc.vector.tensor_sub(tmp[:], possum[:], negsum[:])
                    nc.vector.tensor_add(tw[:], corr[:], corr[:])
                    if b0 <= GRP // 2:
                        nc.vector.tensor_add(res[:, it:it + 1], tmp[:], tw[:])
                    else:
                        nc.vector.tensor_sub(res[:, it:it + 1], tmp[:], tw[:])

            nc.sync.dma_start(o[:], res[:])

    nc.compile()
    return nc


def kernel(x, y_train, alphas, g):
    from concourse.bass_utils import run_bass_kernel_spmd

    x = np.asarray(x, dtype=np.float32)
    y_train = np.asarray(y_train, dtype=np.float32)
    a = np.asarray(alphas, dtype=np.float32).reshape(-1)
    gf = float(np.asarray(g).reshape(-1)[0])

    y2 = np.sum(y_train.astype(np.float64) ** 2, axis=1)
    with np.errstate(divide="ignore"):
        c = -y2 / gf + np.log(np.abs(a.astype(np.float64)))
    c = np.maximum(c, -1e4)

    pos = a >= 0
    order = np.concatenate([np.nonzero(pos)[0], np.nonzero(~pos)[0]])
    npos = int(pos.sum())

    yq = (2.0 / gf) * y_train[order].T.astype(np.float64)   # [DIM, M]
    co = c[order]
    chi64 = co.astype(np.float16).astype(np.float64)
    yhd = yq.astype(np.float16)
    cd = np.empty((2, M), dtype=np.float16)
    cd[0] = chi64.astype(np.float16)
    cd[1] = (co - chi64).astype(np.float16)

    key = npos
    if key not in _cache:
        _cache[key] = _build(npos)
    nc = _cache[key]

    in_maps = []
    for k in range(NCORES):
        xs = x[k * NLOC:(k + 1) * NLOC].T.astype(np.float64)   # [DIM, NLOC]
        xh64 = xs.astype(np.float16).astype(np.float64)
        xpk = np.empty((KPK, NLOC), dtype=np.float16)
        xpk[0:DIM] = xh64.astype(np.float16)
        xpk[DIM:2 * DIM] = (xs - xh64).astype(np.float16)
        xpk[2 * DIM:] = 1.0
        in_maps.append({"yhd": yhd, "cd": cd, "xpk": xpk})

    r = run_bass_kernel_spmd(nc, in_maps, core_ids=list(range(NCORES)))

    x2 = np.sum(x.astype(np.float64) ** 2, axis=1)
    rowscale = np.exp(-x2 / gf)
    out = np.empty(N, dtype=np.float64)
    for k in range(NCORES):
        out[k * NLOC:(k + 1) * NLOC] = r.results[k]["o"].T.reshape(NLOC).astype(np.float64)
    out *= rowscale
    return out.astype(np.float32).reshape(N, 1)
